# revision 2
# baseline (speedup 1.0000x reference)
"""Decoder attention (QKV proj + KV-cache scatter + full softmax attention + out proj)
on 8 Trainium2 cores.

Sharding: core = (batch b, head-group g).  b = core//2, g = core%2; each core
handles 8 of the 16 heads for one batch element.

Softmax + attn@V are invariant to a permutation of the key axis, so the
reference's masked_scatter of new K/V into the cache is equivalent to
attending over concat([k_new, cache_keep]) where cache_keep are the cache
rows NOT in update_idx (complement set, gathered host-side).  No on-device
scatter is needed.

Device kernel (per core), v2 structure:
  - heads are processed in PAIRS: pair hp = heads (2hp, 2hp+1), which live in
    partitions [0:64) and [64:128) of the same q/k tiles.  The two heads'
    score matmuls (contraction = head_dim = 64) run CONCURRENTLY on the PE
    via row tiling (tile_position=(0,0) / (64,0)) -- 2x score throughput vs
    one K=64 matmul at a time.
  - one exp (ACT) instruction per (pair, j, query-chunk) covers both heads'
    scores [128, 1024].  ACT exp is the throughput floor (~1 elem/cycle/lane
    @1.2GHz); all PE work (qkv, v, proj) is drained into PE idle gaps of the
    attention loop via a pending-thunk list.
  - attn@V per head with V augmented by a ones-column -> softmax denominator
    accumulates free in row 64 of the same PSUM tile (M=65).
  - norm: reciprocal + gpsimd partition_broadcast + DVE multiply, straight
    from PSUM (no copy).
  - weights are pre-swizzled host-side (wqkb/wprojb) so each m-tile's column
    block is ONE contiguous [128, .] DMA with 1-2KB lines.

All matmuls bf16 (fp32 PSUM accumulation).
"""

import sys

import os

for _p in ("/opt/trn_rl_repo", "/root/.axon_site/_ro/trn_rl_repo"):
    if os.path.isdir(_p) and _p not in sys.path:
        sys.path.insert(0, _p)
        break

import numpy as np

import concourse.bacc as bacc
import concourse.mybir as mybir
import concourse.tile as tile
from concourse import bass_utils

B, NX, NC, C, H = 4, 1024, 2048, 1024, 16
DH = C // H                      # 64
G = 2                            # head groups (tensor-parallel factor)
HPG = H // G                     # 8 heads per group
CG = HPG * DH                    # 512 channels per group
SCALE = DH ** -0.5
N_CORES = 8
NJ = NC // 128                   # 16 j-tiles over the effective kv length
VW = DH + 1                      # 65: v columns + ones column per head
F32 = mybir.dt.float32
BF16 = mybir.dt.bfloat16
EXP = mybir.ActivationFunctionType.Exp

_STATE = {}


def _build(reps: int = 1):
    """Build + compile the per-core Bass program.

    reps > 1 wraps the whole computation in an on-device hardware loop --
    used only for timing (amortizes host->device dispatch latency).
    """
    nc = bacc.Bacc("TRN2", target_bir_lowering=False, debug=False)

    xT_d = nc.dram_tensor("xT", [C, NX], BF16, kind="ExternalInput")
    wqkb_d = nc.dram_tensor("wqkb", [1024, 1024], BF16, kind="ExternalInput")
    wvT_d = nc.dram_tensor("wvT", [C, CG], BF16, kind="ExternalInput")
    bqk_d = nc.dram_tensor("bqk", [128, 8], F32, kind="ExternalInput")
    bv_d = nc.dram_tensor("bv", [128, CG], F32, kind="ExternalInput")
    kkeepT_d = nc.dram_tensor("kkeepT", [CG, NC - NX], BF16, kind="ExternalInput")
    vkeep_d = nc.dram_tensor("vkeep", [NC - NX, HPG * VW], BF16, kind="ExternalInput")
    wprojb_d = nc.dram_tensor("wprojb", [1024, CG], BF16, kind="ExternalInput")
    ones_d = nc.dram_tensor("ones8", [128, 8], BF16, kind="ExternalInput")
    outT_d = nc.dram_tensor("outT", [C, NX], F32, kind="ExternalOutput")

    with tile.TileContext(nc) as tc:
        with (
            tc.tile_pool(name="persist", bufs=1) as pp,
            tc.tile_pool(name="work", bufs=1) as wp,
            tc.tile_pool(name="attn", bufs=4) as ep,
            tc.tile_pool(name="nrm", bufs=2) as np_pool,
            tc.tile_pool(name="out_sb", bufs=2) as op,
            tc.tile_pool(name="ps", bufs=1, space="PSUM") as psp,
        ):
            # ---- persistent tiles ----
            q_t = [pp.tile([128, NX], BF16, tag=f"q{i}", name=f"q{i}") for i in range(4)]
            k_t = [pp.tile([128, NC], BF16, tag=f"k{i}", name=f"k{i}") for i in range(4)]
            v_t = [pp.tile([128, HPG * VW], BF16, tag=f"v{i}", name=f"v{i}") for i in range(NJ)]
            a_t = [pp.tile([128, NX], BF16, tag=f"a{i}", name=f"a{i}") for i in range(4)]
            bqk_t = pp.tile([128, 8], F32, tag="bqk")
            bv_t = pp.tile([128, CG], F32, tag="bv")
            xT_t = [wp.tile([128, NX], BF16, tag=f"x{i}", name=f"x{i}") for i in range(8)]
            wv_t = [wp.tile([128, CG], BF16, tag=f"wv{i}", name=f"wv{i}") for i in range(8)]
            wqk_t = [wp.tile([128, 1024], BF16, tag=f"wqk{i}", name=f"wqk{i}") for i in range(8)]
            wpb_t = [wp.tile([128, CG], BF16, tag=f"wpb{i}", name=f"wpb{i}") for i in range(8)]

            nc.sync.dma_start(bqk_t[:], bqk_d.ap())
            nc.sync.dma_start(bv_t[:], bv_d.ap())

            def body():
                # ---- DMAs, ordered by first use; big tiles split in halves
                # so two DMA engines work each and first-needed data lands
                # early ----
                for i in range(8):
                    nc.sync.dma_start(xT_t[i][:], xT_d[i * 128:(i + 1) * 128, :])
                for m in (0, 4):
                    for h2 in range(2):
                        nc.sync.dma_start(
                            wqk_t[m][:, h2 * 512:(h2 + 1) * 512],
                            wqkb_d[m * 128:(m + 1) * 128, h2 * 512:(h2 + 1) * 512],
                        )
                # keep-keys for pair 0 first (cache-half j tiles run first)
                for h2 in range(2):
                    nc.sync.dma_start(
                        k_t[0][:, NX + h2 * 512:NX + (h2 + 1) * 512],
                        kkeepT_d[0:128, h2 * 512:(h2 + 1) * 512],
                    )
                for j in range(NJ // 2, NJ):
                    r0 = (j - NJ // 2) * 128
                    nc.sync.dma_start(v_t[j][:], vkeep_d[r0:r0 + 128, :])
                for i in range(1, 4):
                    nc.sync.dma_start(k_t[i][:, NX:NC], kkeepT_d[i * 128:(i + 1) * 128, :])
                # ones columns of the new-token v tiles (bias adds never touch
                # them; vkeep rows arrive with ones baked in)
                for m in range(NJ // 2):
                    nc.sync.dma_start(
                        v_t[m][:].rearrange("p (h w) -> p h w", w=VW)[:, :, DH],
                        ones_d.ap(),
                    )
                for i in range(8):
                    nc.sync.dma_start(wv_t[i][:], wvT_d[i * 128:(i + 1) * 128, :])
                for m in (1, 5, 2, 6, 3, 7):
                    nc.sync.dma_start(wqk_t[m][:], wqkb_d[m * 128:(m + 1) * 128, :])
                for m in range(8):
                    nc.sync.dma_start(wpb_t[m][:], wprojb_d[m * 128:(m + 1) * 128, :])

                # ---- thunk generators (each thunk = 1 PE matmul or 1 DVE
                # finisher); drained into attention PE gaps ----
                def qk_thunks(pair):
                    for m in (pair, 4 + pair):
                        for cch in range(2):
                            qps = psp.tile([128, 512], F32, tag="wps", bufs=2,
                                           name=f"qps{m}_{cch}")
                            for kk in range(8):
                                def mm(m=m, cch=cch, kk=kk, qps=qps):
                                    nc.tensor.matmul(
                                        qps[:],
                                        wqk_t[m][:, kk * 128:(kk + 1) * 128],
                                        xT_t[kk][:, cch * 512:(cch + 1) * 512],
                                        start=(kk == 0),
                                        stop=(kk == 7),
                                    )
                                yield mm
                            def bias(m=m, cch=cch, qps=qps):
                                if m < 4:
                                    dest = q_t[m][:, cch * 512:(cch + 1) * 512]
                                else:
                                    dest = k_t[m - 4][:, cch * 512:(cch + 1) * 512]
                                nc.vector.tensor_scalar_add(dest, qps[:], bqk_t[:, m:m + 1])
                            yield bias

                def v_thunks():
                    for m in range(8):
                        vps = psp.tile([128, 512], F32, tag="wps", bufs=2,
                                       name=f"vps{m}")
                        for kk in range(8):
                            def mm(m=m, kk=kk, vps=vps):
                                nc.tensor.matmul(
                                    vps[:],
                                    xT_t[kk][:, m * 128:(m + 1) * 128],
                                    wv_t[kk][:],
                                    start=(kk == 0),
                                    stop=(kk == 7),
                                )
                            yield mm
                        def fin(m=m, vps=vps):
                            nc.vector.tensor_add(
                                v_t[m][:].rearrange("p (h w) -> p h w", w=VW)[:, :, 0:DH],
                                vps[:].rearrange("p (h w) -> p h w", w=DH),
                                bv_t[:].rearrange("p (h w) -> p h w", w=DH),
                            )
                        yield fin

                def proj_thunks(cch):
                    for m in range(8):
                        pps = psp.tile([128, 512], F32, tag="wps", bufs=2,
                                       name=f"pps{m}_{cch}")
                        for kk in range(4):
                            def mm(m=m, cch=cch, kk=kk, pps=pps):
                                nc.tensor.matmul(
                                    pps[:],
                                    wpb_t[m][:, kk * 128:(kk + 1) * 128],
                                    a_t[kk][:, cch * 512:(cch + 1) * 512],
                                    start=(kk == 0),
                                    stop=(kk == 3),
                                )
                            yield mm
                        def evac(m=m, cch=cch, pps=pps):
                            ot = op.tile([128, 512], F32, tag="ot", bufs=2,
                                         name=f"ot{m}_{cch}")
                            nc.vector.tensor_copy(ot[:], pps[:])
                            nc.sync.dma_start(
                                outT_d[m * 128:(m + 1) * 128, cch * 512:(cch + 1) * 512],
                                ot[:],
                            )
                        yield evac

                pending = []

                def drain(n):
                    for _ in range(n):
                        if not pending:
                            return
                        pending.pop(0)()

                # ---- attention: pair-major blocks, row-tiled scores ----
                def attn_block(hp, cch, j_order, drain_per_j, av_lag=2):
                    gA, gB = 2 * hp, 2 * hp + 1
                    qs = slice(cch * 512, (cch + 1) * 512)
                    avA = psp.tile([VW, 512], F32, tag="avA", bufs=1,
                                   name=f"avA{hp}_{cch}")
                    avB = psp.tile([VW, 512], F32, tag="avB", bufs=1,
                                   name=f"avB{hp}_{cch}")
                    jo = list(j_order)
                    ets = [None] * NJ

                    def emit_av(j):
                        nc.tensor.matmul(
                            avA[:],
                            v_t[j][:, gA * VW:(gA + 1) * VW],
                            ets[j][:, 0:512],
                            start=(j == jo[0]),
                            stop=(j == jo[-1]),
                        )
                        nc.tensor.matmul(
                            avB[:],
                            v_t[j][:, gB * VW:(gB + 1) * VW],
                            ets[j][:, 512:1024],
                            start=(j == jo[0]),
                            stop=(j == jo[-1]),
                        )

                    for step, j in enumerate(jo):
                        sps = psp.tile([128, 1024], F32, tag="sps", bufs=2,
                                       name=f"sps{hp}_{cch}_{j}")
                        nc.tensor.matmul(
                            sps[:, 0:512],
                            k_t[hp][0:64, j * 128:(j + 1) * 128],
                            q_t[hp][0:64, qs],
                            start=True, stop=True,
                            tile_position=(0, 0),
                        )
                        nc.tensor.matmul(
                            sps[:, 512:1024],
                            k_t[hp][64:128, j * 128:(j + 1) * 128],
                            q_t[hp][64:128, qs],
                            start=True, stop=True,
                            tile_position=(64, 0),
                        )
                        et = ep.tile([128, 1024], BF16, tag="et",
                                     name=f"et{hp}_{cch}_{j}")
                        ets[j] = et
                        nc.scalar.activation(et[:], sps[:], EXP, scale=SCALE)
                        drain(drain_per_j)
                        if step >= av_lag:
                            emit_av(jo[step - av_lag])
                    for step in range(NJ - av_lag, NJ):
                        emit_av(jo[step])

                    # normalize straight out of PSUM
                    for g, av, p0 in ((gA, avA, 0), (gB, avB, 64)):
                        recip = np_pool.tile([1, 512], F32, tag="recip", bufs=2,
                                             name=f"rc{g}_{cch}")
                        nc.vector.reciprocal(recip[:], av[DH:VW, :])
                        rb = np_pool.tile([64, 512], F32, tag="rb", bufs=2,
                                          name=f"rb{g}_{cch}")
                        nc.gpsimd.partition_broadcast(rb[:], recip[:])
                        nc.vector.tensor_mul(a_t[hp][p0:p0 + 64, qs], av[0:DH, :], rb[:])

                # qk pair 0 runs up front (attention depends on it)
                for th in qk_thunks(0):
                    th()
                pending.extend(v_thunks())

                cache_first = list(range(NJ // 2, NJ)) + list(range(NJ // 2))
                for bi, (hp, cch) in enumerate(
                    [(0, 0), (0, 1), (1, 0), (1, 1), (2, 0), (2, 1), (3, 0), (3, 1)]
                ):
                    if (hp, cch) == (0, 1):
                        pending.extend(qk_thunks(1))
                    elif (hp, cch) == (1, 1):
                        pending.extend(qk_thunks(2))
                    elif (hp, cch) == (2, 1):
                        pending.extend(qk_thunks(3))
                    elif (hp, cch) == (3, 1):
                        pending.extend(proj_thunks(0))
                    jo = cache_first if bi < 2 else range(NJ)
                    attn_block(hp, cch, jo, drain_per_j=6 if bi < 2 else 4)
                drain(len(pending))

                for th in proj_thunks(1):
                    th()

            if reps == 1:
                body()
            else:
                hints = (
                    mybir.EngineType.PE,
                    mybir.EngineType.Activation,
                    mybir.EngineType.DVE,
                    mybir.EngineType.SP,
                )
                with tc.For_i(0, reps, 1, hint_engines=hints):
                    body()

    nc.compile()
    return nc


def _get_nc():
    if "nc" not in _STATE:
        _STATE["nc"] = _build()
    return _STATE["nc"]


def _prep_in_maps(x, update_idx, cache_k, cache_v, w_qkv, b_qkv):
    """Host-side sharding: build the 8 per-core input dicts."""
    import ml_dtypes

    x = np.asarray(x, np.float32)
    update_idx = np.asarray(update_idx)
    cache_k = np.asarray(cache_k, np.float32)
    cache_v = np.asarray(cache_v, np.float32)
    w_qkv = np.asarray(w_qkv, np.float32)
    b_qkv = np.asarray(b_qkv, np.float32)

    per_g = []
    for g in range(G):
        qs = slice(g * CG, (g + 1) * CG)
        ks = slice(C + g * CG, C + (g + 1) * CG)
        vs = slice(2 * C + g * CG, 2 * C + (g + 1) * CG)
        wqkT = np.ascontiguousarray(
            np.concatenate([w_qkv[qs], w_qkv[ks]], 0).T
        )                                                    # (C, 2CG)
        # swizzle: wqkb[m*128+p, kk*128+f] = wqkT[kk*128+p, m*128+f]
        wqkb = np.ascontiguousarray(
            wqkT.reshape(8, 128, 8, 128).transpose(2, 1, 0, 3).reshape(1024, 1024)
        ).astype(ml_dtypes.bfloat16)
        wvT = np.ascontiguousarray(w_qkv[vs].T).astype(ml_dtypes.bfloat16)  # (C, CG)
        bqk = np.ascontiguousarray(
            np.concatenate([b_qkv[qs], b_qkv[ks]]).reshape(8, 128).T
        )                                                    # (128, 8)
        bv = np.broadcast_to(b_qkv[vs][None, :], (128, CG)).copy()
        wprojT = np.asarray(_STATE["wprojT"][g], np.float32)  # (CG, C)
        wprojb = np.ascontiguousarray(
            wprojT.reshape(4, 128, 8, 128).transpose(2, 1, 0, 3).reshape(1024, CG)
        ).astype(ml_dtypes.bfloat16)
        per_g.append((wqkb, wvT, bqk, bv, wprojb))

    in_maps = []
    for b in range(B):
        idx = update_idx[b]
        mask = np.ones(NC, bool)
        mask[idx] = False
        keep = np.nonzero(mask)[0]                           # (NC-NX,) sorted
        xT = np.ascontiguousarray(x[b].T).astype(ml_dtypes.bfloat16)  # (C, NX)
        for g in range(G):
            wqkb, wvT, bqk, bv, wprojb = per_g[g]
            hsel = slice(g * HPG, (g + 1) * HPG)
            kk = cache_k[b, hsel][:, keep, :]                # (HPG, NC-NX, DH)
            kkeepT = np.ascontiguousarray(
                kk.transpose(0, 2, 1).reshape(HPG * DH, NC - NX)
            ).astype(ml_dtypes.bfloat16)
            vk = cache_v[b, hsel][:, keep, :].transpose(1, 0, 2)  # (NC-NX, HPG, DH)
            vkeep = np.ascontiguousarray(
                np.concatenate(
                    [vk, np.ones((NC - NX, HPG, 1), np.float32)], axis=2
                ).reshape(NC - NX, HPG * VW)
            ).astype(ml_dtypes.bfloat16)
            in_maps.append(
                dict(
                    xT=xT, wqkb=wqkb, wvT=wvT, bqk=bqk, bv=bv,
                    kkeepT=kkeepT, vkeep=vkeep, wprojb=wprojb,
                    ones8=np.ones((128, 8), ml_dtypes.bfloat16),
                )
            )
    return in_maps


def kernel(x, update_idx, cache_k, cache_v, w_qkv, b_qkv, w_proj, b_proj):
    nc = _get_nc()
    w_proj = np.asarray(w_proj, np.float32)
    b_proj = np.asarray(b_proj, np.float32)
    _STATE["wprojT"] = [
        np.ascontiguousarray(w_proj[:, g * CG:(g + 1) * CG].T) for g in range(G)
    ]
    in_maps = _prep_in_maps(x, update_idx, cache_k, cache_v, w_qkv, b_qkv)
    res = bass_utils.run_bass_kernel_spmd(nc, in_maps, core_ids=list(range(N_CORES)))
    _STATE["last_results"] = res
    out = np.empty((B, NX, C), np.float32)
    for b in range(B):
        acc = res.results[2 * b]["outT"] + res.results[2 * b + 1]["outT"]
        out[b] = acc.T + b_proj
    return out


# revision 27
# speedup vs baseline: 1.2073x; 1.2073x over previous
"""Decoder attention (QKV proj + KV-cache scatter + full softmax attention + out proj)
on 8 Trainium2 cores.

Sharding: core = (batch b, head-group g).  b = core//2, g = core%2; each core
handles 8 of the 16 heads for one batch element.

Softmax + attn@V are invariant to a permutation of the key axis, so the
reference's masked_scatter of new K/V into the cache is equivalent to
attending over concat([k_new, cache_keep]) where cache_keep are the cache
rows NOT in update_idx (complement set, gathered host-side).  No on-device
scatter is needed.

Device kernel (per core), final structure (variant "fp3",
HW-measured 239.8us/iter vs the 307.6us baseline; every choice below was
A/B-measured on hardware via the loop-slope method):
  - heads are processed in PAIRS: pair hp = heads (2hp, 2hp+1), which live in
    partitions [0:64) and [64:128) of the same q/k tiles.  The two heads'
    score matmuls (contraction = head_dim = 64) run CONCURRENTLY on the PE
    via row tiling (tile_position=(0,0) / (64,0)); serializing them (probe)
    costs +51.7us/iter, so the concurrency (and the LDWEIGHTS row-group
    alternation it enables) is the single biggest win.
  - one exp (ACT) instruction per (pair, j, query-chunk) covers both heads'
    scores [128, 1024]; emission is grouped in 2-j steps (4 score MMs, 2
    exps, drains, 4 av MMs) to halve cross-engine handoff frequency.
  - attn@V per head with V augmented by a ones-column -> softmax denominator
    accumulates free in row 64 of the same PSUM tile (M=65).
  - norm is OFF the critical path: av PSUM is evacuated to SBUF first (frees
    the bank for the next block's accumulation), then 1/denom + gpsimd
    partition_broadcast + DVE multiply trail lazily.  (The stock
    nc.vector.reciprocal is ~6 cycles/elem; keeping it off-path was worth
    ~30us.  reciprocal_approx_fast would be faster still but the custom-DVE
    uop table does not reach the HW through this axon/pjrt path -- produces
    garbage; measured and rejected.)
  - qkv/v/out-proj matmuls are drained one-per-attention-step into PE gaps
    via a pending-thunk FIFO; proj(cch0) drains inside the last attention
    block, proj(cch1) tails.
  - weights are pre-swizzled host-side (wqkb/wprojb) so each m-tile's column
    block is ONE contiguous [128, .] DMA with 1-2KB lines; v-tile ones
    columns are DVE memsets (not DMAs); big kkeep DMAs are split in halves
    across DMA queues.

All matmuls bf16 (fp32 PSUM accumulation).  PSUM budget (8 banks): sps
[128,1024] x2 bufs (4) + avA/avB [65,512] (2) + work tile [128,512] x2 (2).
"""

import sys

import os

for _p in ("/opt/trn_rl_repo", "/root/.axon_site/_ro/trn_rl_repo"):
    if os.path.isdir(_p) and _p not in sys.path:
        sys.path.insert(0, _p)
        break

import numpy as np

import concourse.bacc as bacc
import concourse.mybir as mybir
import concourse.tile as tile
from concourse import bass_utils

B, NX, NC, C, H = 4, 1024, 2048, 1024, 16
DH = C // H                      # 64
G = 2                            # head groups (tensor-parallel factor)
HPG = H // G                     # 8 heads per group
CG = HPG * DH                    # 512 channels per group
SCALE = DH ** -0.5
N_CORES = 8
NJ = NC // 128                   # 16 j-tiles over the effective kv length
VW = DH + 1                      # 65: v columns + ones column per head
F32 = mybir.dt.float32
BF16 = mybir.dt.bfloat16
EXP = mybir.ActivationFunctionType.Exp

_STATE = {}


def _build(reps: int = 1, variant: str = "fp3"):
    """Build + compile the per-core Bass program.

    reps > 1 wraps the whole computation in an on-device hardware loop --
    used only for timing (amortizes host->device dispatch latency).
    variant: "" (real kernel) or a timing-only probe:
      "noexp"  - av matmuls read a constant garbage et tile (no ACT dep)
      "noattn" - skip the attention j-loops entirely (qkv+v+proj+DMA only)
      "noav"   - scores+exp only, no attn@V matmuls
    """
    nc = bacc.Bacc("TRN2", target_bir_lowering=False, debug=False)

    xT_d = nc.dram_tensor("xT", [C, NX], BF16, kind="ExternalInput")
    wqkb_d = nc.dram_tensor("wqkb", [1024, 1024], BF16, kind="ExternalInput")
    wvT_d = nc.dram_tensor("wvT", [C, CG], BF16, kind="ExternalInput")
    bqk_d = nc.dram_tensor("bqk", [128, 8], F32, kind="ExternalInput")
    bv_d = nc.dram_tensor("bv", [128, CG], F32, kind="ExternalInput")
    kkeepT_d = nc.dram_tensor("kkeepT", [CG, NC - NX], BF16, kind="ExternalInput")
    vkeep_d = nc.dram_tensor("vkeep", [NC - NX, HPG * VW], BF16, kind="ExternalInput")
    wprojb_d = nc.dram_tensor("wprojb", [1024, CG], BF16, kind="ExternalInput")
    ones_d = nc.dram_tensor("ones8", [128, 8], BF16, kind="ExternalInput")
    outT_d = nc.dram_tensor("outT", [C, NX], F32, kind="ExternalOutput")

    with tile.TileContext(nc) as tc:
        with (
            tc.tile_pool(name="persist", bufs=1) as pp,
            tc.tile_pool(name="work", bufs=1) as wp,
            tc.tile_pool(name="attn", bufs=4) as ep,
            tc.tile_pool(name="nrm", bufs=2) as np_pool,
            tc.tile_pool(name="out_sb", bufs=2) as op,
            tc.tile_pool(name="ps", bufs=1, space="PSUM") as psp,
        ):
            # ---- persistent tiles ----
            q_t = [pp.tile([128, NX], BF16, tag=f"q{i}", name=f"q{i}") for i in range(4)]
            k_t = [pp.tile([128, NC], BF16, tag=f"k{i}", name=f"k{i}") for i in range(4)]
            v_t = [pp.tile([128, HPG * VW], BF16, tag=f"v{i}", name=f"v{i}") for i in range(NJ)]
            a_t = [pp.tile([128, NX], BF16, tag=f"a{i}", name=f"a{i}") for i in range(4)]
            bqk_t = pp.tile([128, 8], F32, tag="bqk")
            bv_t = pp.tile([128, CG], F32, tag="bv")
            xT_t = [wp.tile([128, NX], BF16, tag=f"x{i}", name=f"x{i}") for i in range(8)]
            wv_t = [wp.tile([128, CG], BF16, tag=f"wv{i}", name=f"wv{i}") for i in range(8)]
            wqk_t = [wp.tile([128, 1024], BF16, tag=f"wqk{i}", name=f"wqk{i}") for i in range(8)]
            wpb_t = [wp.tile([128, CG], BF16, tag=f"wpb{i}", name=f"wpb{i}") for i in range(8)]

            nc.sync.dma_start(bqk_t[:], bqk_d.ap())
            nc.sync.dma_start(bv_t[:], bv_d.ap())

            def body():
                # ---- DMAs, ordered by first use; big tiles split in halves
                # so two DMA engines work each and first-needed data lands
                # early ----
                for i in range(8):
                    nc.sync.dma_start(xT_t[i][:], xT_d[i * 128:(i + 1) * 128, :])
                for m in (0, 4):
                    for h2 in range(2):
                        nc.sync.dma_start(
                            wqk_t[m][:, h2 * 512:(h2 + 1) * 512],
                            wqkb_d[m * 128:(m + 1) * 128, h2 * 512:(h2 + 1) * 512],
                        )
                # keep-keys for pair 0 first (cache-half j tiles run first)
                for h2 in range(2):
                    nc.sync.dma_start(
                        k_t[0][:, NX + h2 * 512:NX + (h2 + 1) * 512],
                        kkeepT_d[0:128, h2 * 512:(h2 + 1) * 512],
                    )
                for j in range(NJ // 2, NJ):
                    r0 = (j - NJ // 2) * 128
                    nc.sync.dma_start(v_t[j][:], vkeep_d[r0:r0 + 128, :])
                if variant == "fp3":
                    for i in range(1, 4):
                        for h2 in range(2):
                            nc.sync.dma_start(
                                k_t[i][:, NX + h2 * 512:NX + (h2 + 1) * 512],
                                kkeepT_d[i * 128:(i + 1) * 128, h2 * 512:(h2 + 1) * 512],
                            )
                else:
                    for i in range(1, 4):
                        nc.sync.dma_start(k_t[i][:, NX:NC], kkeepT_d[i * 128:(i + 1) * 128, :])
                # ones columns of the new-token v tiles (bias adds never touch
                # them; vkeep rows arrive with ones baked in)
                if variant == "fp3":
                    for m in range(NJ // 2):
                        nc.vector.memset(
                            v_t[m][:].rearrange("p (h w) -> p h w", w=VW)[:, :, DH],
                            1.0,
                        )
                else:
                    for m in range(NJ // 2):
                        nc.sync.dma_start(
                            v_t[m][:].rearrange("p (h w) -> p h w", w=VW)[:, :, DH],
                            ones_d.ap(),
                        )
                for i in range(8):
                    nc.sync.dma_start(wv_t[i][:], wvT_d[i * 128:(i + 1) * 128, :])
                for m in (1, 5, 2, 6, 3, 7):
                    nc.sync.dma_start(wqk_t[m][:], wqkb_d[m * 128:(m + 1) * 128, :])
                for m in range(8):
                    nc.sync.dma_start(wpb_t[m][:], wprojb_d[m * 128:(m + 1) * 128, :])

                # ---- thunk generators (each thunk = 1 PE matmul or 1 DVE
                # finisher); drained into attention PE gaps ----
                def qk_thunks(pair):
                    if variant == "qkvdedup":
                        # adjacent same-stationary matmul pairs (cch0+cch1)
                        for m in (pair, 4 + pair):
                            qps = psp.tile([128, 1024], F32, tag="wpsbig", bufs=1,
                                           name=f"qps{m}")
                            for kk in range(8):
                                def mm(m=m, kk=kk, qps=qps):
                                    for cch in range(2):
                                        nc.tensor.matmul(
                                            qps[:, cch * 512:(cch + 1) * 512],
                                            wqk_t[m][:, kk * 128:(kk + 1) * 128],
                                            xT_t[kk][:, cch * 512:(cch + 1) * 512],
                                            start=(kk == 0),
                                            stop=(kk == 7),
                                        )
                                yield mm
                            def bias(m=m, qps=qps):
                                if m < 4:
                                    dest = q_t[m][:]
                                else:
                                    dest = k_t[m - 4][:, 0:NX]
                                nc.vector.tensor_scalar_add(dest, qps[:], bqk_t[:, m:m + 1])
                            yield bias
                        return
                    for m in (pair, 4 + pair):
                        for cch in range(2):
                            qps = psp.tile([128, 512], F32, tag="wps", bufs=2,
                                           name=f"qps{m}_{cch}")
                            for kk in range(8):
                                def mm(m=m, cch=cch, kk=kk, qps=qps):
                                    nc.tensor.matmul(
                                        qps[:],
                                        wqk_t[m][:, kk * 128:(kk + 1) * 128],
                                        xT_t[kk][:, cch * 512:(cch + 1) * 512],
                                        start=(kk == 0),
                                        stop=(kk == 7),
                                    )
                                yield mm
                            def bias(m=m, cch=cch, qps=qps):
                                if m < 4:
                                    dest = q_t[m][:, cch * 512:(cch + 1) * 512]
                                else:
                                    dest = k_t[m - 4][:, cch * 512:(cch + 1) * 512]
                                nc.vector.tensor_scalar_add(dest, qps[:], bqk_t[:, m:m + 1])
                            yield bias

                def v_thunks():
                    wtag, wbufs = (("wpsbig", 1) if variant == "qkvdedup"
                                   else ("wps", 2))
                    for m in range(8):
                        vps = psp.tile([128, 512], F32, tag=wtag, bufs=wbufs,
                                       name=f"vps{m}")
                        for kk in range(8):
                            def mm(m=m, kk=kk, vps=vps):
                                nc.tensor.matmul(
                                    vps[:],
                                    xT_t[kk][:, m * 128:(m + 1) * 128],
                                    wv_t[kk][:],
                                    start=(kk == 0),
                                    stop=(kk == 7),
                                )
                            yield mm
                        def fin(m=m, vps=vps):
                            nc.vector.tensor_add(
                                v_t[m][:].rearrange("p (h w) -> p h w", w=VW)[:, :, 0:DH],
                                vps[:].rearrange("p (h w) -> p h w", w=DH),
                                bv_t[:].rearrange("p (h w) -> p h w", w=DH),
                            )
                        yield fin

                def proj_thunks(cch):
                    wtag, wbufs = (("wpsbig", 1) if variant == "qkvdedup"
                                   else ("wps", 2))
                    for m in range(8):
                        pps = psp.tile([128, 512], F32, tag=wtag, bufs=wbufs,
                                       name=f"pps{m}_{cch}")
                        for kk in range(4):
                            def mm(m=m, cch=cch, kk=kk, pps=pps):
                                nc.tensor.matmul(
                                    pps[:],
                                    wpb_t[m][:, kk * 128:(kk + 1) * 128],
                                    a_t[kk][:, cch * 512:(cch + 1) * 512],
                                    start=(kk == 0),
                                    stop=(kk == 3),
                                )
                            yield mm
                        def evac(m=m, cch=cch, pps=pps):
                            ot = op.tile([128, 512], F32, tag="ot", bufs=2,
                                         name=f"ot{m}_{cch}")
                            nc.vector.tensor_copy(ot[:], pps[:])
                            eng = nc.gpsimd if variant == "fp2" else nc.sync
                            eng.dma_start(
                                outT_d[m * 128:(m + 1) * 128, cch * 512:(cch + 1) * 512],
                                ot[:],
                            )
                        yield evac

                pending = []

                def drain(n):
                    for _ in range(n):
                        if not pending:
                            return
                        pending.pop(0)()

                et_const = None
                if variant in ("noexp", "fpnx"):
                    et_const = ep.tile([128, 1024], BF16, tag="etc", name="et_const")
                    nc.vector.memset(et_const[:], 1.0)
                if variant == "noattn":
                    for t in a_t:
                        nc.vector.memset(t[:], 0.5)

                # ---- attention: pair-major blocks, row-tiled scores ----
                def attn_block(hp, cch, j_order, drain_per_j, av_lag=2):
                    gA, gB = 2 * hp, 2 * hp + 1
                    qs = slice(cch * 512, (cch + 1) * 512)
                    avA = psp.tile([VW, 512], F32, tag="avA", bufs=1,
                                   name=f"avA{hp}_{cch}")
                    avB = psp.tile([VW, 512], F32, tag="avB", bufs=1,
                                   name=f"avB{hp}_{cch}")
                    jo = list(j_order)
                    ets = [None] * NJ

                    def emit_av(j):
                        nc.tensor.matmul(
                            avA[:],
                            v_t[j][:, gA * VW:(gA + 1) * VW],
                            ets[j][:, 0:512],
                            start=(j == jo[0]),
                            stop=(j == jo[-1]),
                        )
                        nc.tensor.matmul(
                            avB[:],
                            v_t[j][:, gB * VW:(gB + 1) * VW],
                            ets[j][:, 512:1024],
                            start=(j == jo[0]),
                            stop=(j == jo[-1]),
                        )

                    if variant in ("pairj", "fp", "fp2", "fp3", "fpnx", "lz", "fr2", "actr"):
                        # emit in 2-j groups: 4 score MMs, 2 exps, drains,
                        # 4 av MMs -- halves cross-engine handoff frequency
                        def emit_scores(j):
                            sps = psp.tile([128, 1024], F32, tag="sps", bufs=2,
                                           name=f"sps{hp}_{cch}_{j}")
                            nc.tensor.matmul(
                                sps[:, 0:512],
                                k_t[hp][0:64, j * 128:(j + 1) * 128],
                                q_t[hp][0:64, qs],
                                start=True, stop=True, tile_position=(0, 0),
                            )
                            nc.tensor.matmul(
                                sps[:, 512:1024],
                                k_t[hp][64:128, j * 128:(j + 1) * 128],
                                q_t[hp][64:128, qs],
                                start=True, stop=True, tile_position=(64, 0),
                            )
                            return sps

                        for g in range(NJ // 2):
                            j0, j1 = jo[2 * g], jo[2 * g + 1]
                            sps0 = emit_scores(j0)
                            sps1 = emit_scores(j1)
                            for j, sps in ((j0, sps0), (j1, sps1)):
                                if variant == "fpnx":
                                    ets[j] = et_const
                                    continue
                                et = ep.tile([128, 1024], BF16, tag="et", bufs=4,
                                             name=f"et{hp}_{cch}_{j}")
                                ets[j] = et
                                nc.scalar.activation(et[:], sps[:], EXP, scale=SCALE)
                            if variant == "fp3" and g >= 1:
                                emit_av(jo[2 * g - 2])
                                emit_av(jo[2 * g - 1])
                            drain(2 * drain_per_j)
                            if variant != "fp3" and g >= 1:
                                emit_av(jo[2 * g - 2])
                                emit_av(jo[2 * g - 1])
                        emit_av(jo[NJ - 2])
                        emit_av(jo[NJ - 1])
                        for g, av, p0 in ((gA, avA, 0), (gB, avB, 64)):
                            if variant in ("fp2", "lz", "fp3", "fr2", "actr"):
                                # evac PSUM first (frees av bank for the next
                                # block), then normalize lazily from SBUF
                                avs = np_pool.tile([VW, 512], F32, tag="avs",
                                                   bufs=4, name=f"avs{g}_{cch}")
                                nc.vector.tensor_copy(avs[:], av[:])
                                src_av = avs
                            else:
                                src_av = av
                            rb = np_pool.tile([64, 512], F32, tag="rb", bufs=2,
                                              name=f"rb{g}_{cch}")
                            if variant == "fr2":
                                db = np_pool.tile([64, 512], F32, tag="db", bufs=2,
                                                  name=f"db{g}_{cch}")
                                nc.gpsimd.partition_broadcast(db[:], src_av[DH:VW, :])
                                nc.vector.reciprocal_approx_fast(rb[:], db[:])
                            else:
                                recip = np_pool.tile([1, 512], F32, tag="recip", bufs=2,
                                                     name=f"rc{g}_{cch}")
                                if variant == "actr":
                                    se = nc.scalar
                                    ins_ = [se.lower_ap(src_av[DH:VW, :])]
                                    for val in (0.0, 1.0, 0.0):
                                        ins_.append(mybir.ImmediateValue(
                                            dtype=mybir.dt.float32, value=val))
                                    se.add_instruction(mybir.InstActivation(
                                        name=nc.get_next_instruction_name(),
                                        func=mybir.ActivationFunctionType.Reciprocal,
                                        ins=ins_, outs=[se.lower_ap(recip[:])]))
                                elif variant in ("pairj", "lz", "fp3"):
                                    nc.vector.reciprocal(recip[:], src_av[DH:VW, :])
                                else:
                                    nc.vector.reciprocal_approx_fast(recip[:], src_av[DH:VW, :])
                                nc.gpsimd.partition_broadcast(rb[:], recip[:])
                            nc.vector.tensor_mul(a_t[hp][p0:p0 + 64, qs], src_av[0:DH, :], rb[:])
                        return

                    for step, j in enumerate(jo):
                        sps = psp.tile([128, 1024], F32, tag="sps", bufs=2,
                                       name=f"sps{hp}_{cch}_{j}")
                        nc.tensor.matmul(
                            sps[:, 0:512],
                            k_t[hp][0:64, j * 128:(j + 1) * 128],
                            q_t[hp][0:64, qs],
                            start=True, stop=True,
                            tile_position=(0, 0),
                        )
                        if variant == "serial_scores":
                            # timing probe: same row-group as mmA (reads head
                            # A's data; results wrong for head B)
                            nc.tensor.matmul(
                                sps[:, 512:1024],
                                k_t[hp][0:64, j * 128:(j + 1) * 128],
                                q_t[hp][0:64, qs],
                                start=True, stop=True,
                                tile_position=(0, 0),
                            )
                        else:
                            nc.tensor.matmul(
                                sps[:, 512:1024],
                                k_t[hp][64:128, j * 128:(j + 1) * 128],
                                q_t[hp][64:128, qs],
                                start=True, stop=True,
                                tile_position=(64, 0),
                            )
                        if variant == "noexp":
                            ets[j] = et_const
                        else:
                            et = ep.tile([128, 1024], BF16, tag="et",
                                         bufs=7 if variant == "avlag4" else 4,
                                         name=f"et{hp}_{cch}_{j}")
                            ets[j] = et
                            nc.scalar.activation(et[:], sps[:], EXP, scale=SCALE)
                        drain(drain_per_j)
                        if variant != "noav" and step >= av_lag:
                            emit_av(jo[step - av_lag])
                    if variant != "noav":
                        for step in range(NJ - av_lag, NJ):
                            emit_av(jo[step])

                    # normalize straight out of PSUM
                    for g, av, p0 in ((gA, avA, 0), (gB, avB, 64)):
                        recip = np_pool.tile([1, 512], F32, tag="recip", bufs=2,
                                             name=f"rc{g}_{cch}")
                        if variant in ("frecip", "fp"):
                            nc.vector.reciprocal_approx_fast(recip[:], av[DH:VW, :])
                        else:
                            nc.vector.reciprocal(recip[:], av[DH:VW, :])
                        rb = np_pool.tile([64, 512], F32, tag="rb", bufs=2,
                                          name=f"rb{g}_{cch}")
                        nc.gpsimd.partition_broadcast(rb[:], recip[:])
                        nc.vector.tensor_mul(a_t[hp][p0:p0 + 64, qs], av[0:DH, :], rb[:])

                # qk pair 0 runs up front (attention depends on it)
                for th in qk_thunks(0):
                    th()
                pending.extend(v_thunks())

                if variant == "v3":
                    extends = {0: [1], 1: [2], 2: [3], 7: ["p0"]}
                    dpj = [5, 4, 3, 3, 3, 3, 2, 3]
                else:
                    extends = {1: [1], 3: [2], 5: [3], 7: ["p0"]}
                    dpj = [6, 6, 4, 4, 4, 4, 4, 4]

                cache_first = list(range(NJ // 2, NJ)) + list(range(NJ // 2))
                for bi, (hp, cch) in enumerate(
                    [(0, 0), (0, 1), (1, 0), (1, 1), (2, 0), (2, 1), (3, 0), (3, 1)]
                ):
                    for ext in extends.get(bi, []):
                        if ext == "p0":
                            pending.extend(proj_thunks(0))
                        else:
                            pending.extend(qk_thunks(ext))
                    jo = cache_first if bi < 2 else range(NJ)
                    if variant != "noattn":
                        attn_block(hp, cch, jo, drain_per_j=dpj[bi],
                                   av_lag=4 if variant == "avlag4" else 2)
                drain(len(pending))

                for th in proj_thunks(1):
                    th()

            if reps == 1:
                body()
            else:
                hints = (
                    mybir.EngineType.PE,
                    mybir.EngineType.Activation,
                    mybir.EngineType.DVE,
                    mybir.EngineType.SP,
                )
                with tc.For_i(0, reps, 1, hint_engines=hints):
                    body()

    nc.compile()
    return nc


def _get_nc():
    if "nc" not in _STATE:
        _STATE["nc"] = _build()
    return _STATE["nc"]


def _prep_in_maps(x, update_idx, cache_k, cache_v, w_qkv, b_qkv):
    """Host-side sharding: build the 8 per-core input dicts."""
    import ml_dtypes

    x = np.asarray(x, np.float32)
    update_idx = np.asarray(update_idx)
    cache_k = np.asarray(cache_k, np.float32)
    cache_v = np.asarray(cache_v, np.float32)
    w_qkv = np.asarray(w_qkv, np.float32)
    b_qkv = np.asarray(b_qkv, np.float32)

    per_g = []
    for g in range(G):
        qs = slice(g * CG, (g + 1) * CG)
        ks = slice(C + g * CG, C + (g + 1) * CG)
        vs = slice(2 * C + g * CG, 2 * C + (g + 1) * CG)
        wqkT = np.ascontiguousarray(
            np.concatenate([w_qkv[qs], w_qkv[ks]], 0).T
        )                                                    # (C, 2CG)
        # swizzle: wqkb[m*128+p, kk*128+f] = wqkT[kk*128+p, m*128+f]
        wqkb = np.ascontiguousarray(
            wqkT.reshape(8, 128, 8, 128).transpose(2, 1, 0, 3).reshape(1024, 1024)
        ).astype(ml_dtypes.bfloat16)
        wvT = np.ascontiguousarray(w_qkv[vs].T).astype(ml_dtypes.bfloat16)  # (C, CG)
        bqk = np.ascontiguousarray(
            np.concatenate([b_qkv[qs], b_qkv[ks]]).reshape(8, 128).T
        )                                                    # (128, 8)
        bv = np.broadcast_to(b_qkv[vs][None, :], (128, CG)).copy()
        wprojT = np.asarray(_STATE["wprojT"][g], np.float32)  # (CG, C)
        wprojb = np.ascontiguousarray(
            wprojT.reshape(4, 128, 8, 128).transpose(2, 1, 0, 3).reshape(1024, CG)
        ).astype(ml_dtypes.bfloat16)
        per_g.append((wqkb, wvT, bqk, bv, wprojb))

    in_maps = []
    for b in range(B):
        idx = update_idx[b]
        mask = np.ones(NC, bool)
        mask[idx] = False
        keep = np.nonzero(mask)[0]                           # (NC-NX,) sorted
        xT = np.ascontiguousarray(x[b].T).astype(ml_dtypes.bfloat16)  # (C, NX)
        for g in range(G):
            wqkb, wvT, bqk, bv, wprojb = per_g[g]
            hsel = slice(g * HPG, (g + 1) * HPG)
            kk = cache_k[b, hsel][:, keep, :]                # (HPG, NC-NX, DH)
            kkeepT = np.ascontiguousarray(
                kk.transpose(0, 2, 1).reshape(HPG * DH, NC - NX)
            ).astype(ml_dtypes.bfloat16)
            vk = cache_v[b, hsel][:, keep, :].transpose(1, 0, 2)  # (NC-NX, HPG, DH)
            vkeep = np.ascontiguousarray(
                np.concatenate(
                    [vk, np.ones((NC - NX, HPG, 1), np.float32)], axis=2
                ).reshape(NC - NX, HPG * VW)
            ).astype(ml_dtypes.bfloat16)
            in_maps.append(
                dict(
                    xT=xT, wqkb=wqkb, wvT=wvT, bqk=bqk, bv=bv,
                    kkeepT=kkeepT, vkeep=vkeep, wprojb=wprojb,
                    ones8=np.ones((128, 8), ml_dtypes.bfloat16),
                )
            )
    return in_maps


def kernel(x, update_idx, cache_k, cache_v, w_qkv, b_qkv, w_proj, b_proj):
    nc = _get_nc()
    w_proj = np.asarray(w_proj, np.float32)
    b_proj = np.asarray(b_proj, np.float32)
    _STATE["wprojT"] = [
        np.ascontiguousarray(w_proj[:, g * CG:(g + 1) * CG].T) for g in range(G)
    ]
    in_maps = _prep_in_maps(x, update_idx, cache_k, cache_v, w_qkv, b_qkv)
    res = bass_utils.run_bass_kernel_spmd(nc, in_maps, core_ids=list(range(N_CORES)))
    _STATE["last_results"] = res
    out = np.empty((B, NX, C), np.float32)
    for b in range(B):
        acc = res.results[2 * b]["outT"] + res.results[2 * b + 1]["outT"]
        out[b] = acc.T + b_proj
    return out
